# revision 30
# baseline (speedup 1.0000x reference)
"""Chunked (= full, non-causal) multi-head self-attention on 8 TRN2 NeuronCores.

Problem: B=2, S=2048, D=1024, H=16 heads (head_dim 64), torch-Linear-style
projections (y = x @ W.T + b), softmax attention, output projection.

Sharding: head-parallel. Core c owns heads {2c, 2c+1} = feature slice
[128c, 128c+128). Each core computes q/k/v for its slice from the full x
(replicated), runs attention for its 4 (batch, head) pairs, and produces a
partial output projection with its 128-row slice of Wo. Host sums the 8
partials and adds bo.

Layout: scores are computed transposed, ST[k, q] (keys on partitions), so the
softmax exp output PT feeds the P@V matmul directly (contraction over k on
partitions) with no on-chip transposes anywhere — x and the weights are
pre-transposed on the host. The two heads' K=64 score matmuls land on PE
row-groups 0-1/2-3. The softmax denominator rides as row 64 of the PV output
via a ones-column appended to V (M=65); normalization is a reciprocal +
rank-1 broadcast matmul + one DVE multiply on the small [64, S] output,
deferred one iteration so it never stalls the PE.

Scheduling: the kernel is emitted as one software-pipelined stream designed to
keep the PE densely busy (TRN2's HAM clock-gate halves the PE clock if it ever
looks idle): projections for batch 0 first, then batch-0 attention interleaved
with batch-1 projections, then batch-1 attention interleaved with the deferred
output projections. Score tiles are written in 2-bank PSUM pairs so each exp
ACTIVATE covers 1024 columns, amortizing ACT's fixed PSUM-access latency.

Precision: everything bf16 in, fp32 accumulate/out (~5e-3 rel err).
"""

import sys

if "/opt/trn_rl_repo" not in sys.path:
    sys.path.insert(0, "/opt/trn_rl_repo")

import numpy as np

import concourse.bacc as bacc
import concourse.mybir as mybir
import concourse.tile as tile
from concourse import bass_utils

# Route Exp to the activation-table set that also holds Ln, so the softmax
# exps and the reciprocal-via-exp(-ln(x)) trick share one table (the default
# per-function choice would ping-pong table loads at ~2.7us each).
_orig_get_activation_tables = bacc.get_activation_tables


def _patched_get_activation_tables(arch):
    out = {}
    for name, funcs in dict(_orig_get_activation_tables(arch)).items():
        if name != "natural_log_exp_and_others":
            funcs = {f for f in funcs if f != mybir.ActivationFunctionType.Exp}
        out[name] = funcs
    return out


bacc.get_activation_tables = _patched_get_activation_tables

B, S, D, H = 2, 2048, 1024, 16
HD = D // H          # 64
NCORES = 8
ES = D // NCORES     # 128 features (= 2 heads) per core
BS = B * S           # 4096 rows total

P = 128              # partitions
NF = 512             # matmul free-dim tile
N_SB = BS // NF      # 8 s-blocks of 512
N_DC = D // P        # 8 contraction chunks of 128
N_KB = S // P        # 16 key blocks of 128 per batch
N_KP = N_KB // 2     # 8 key-block PAIRS per batch
N_QC = S // NF       # 4 query chunks of 512 per batch
N_CH = BS // P       # 32 global 128-row chunks

F32 = mybir.dt.float32
BF16 = mybir.dt.bfloat16

DT_QK = BF16         # x/Wq/Wk inputs for q,k projections + score matmuls
DT_V = BF16          # x/Wv inputs for v projection
DT_ATT = BF16        # attention weights (exp output) and V in the P@V matmul
DT_OUT = BF16        # output projection inputs (OT, Wo)

DEBUG = False

_cache = {}
last_results = None          # test.py reads exec_time_ns off this


def _np_dt(dt):
    import ml_dtypes

    return np.dtype(ml_dtypes.bfloat16) if dt == mybir.dt.bfloat16 else np.dtype(np.float32)


def _build():
    nc = bacc.Bacc("TRN2", target_bir_lowering=False, debug=False)

    xT_d = nc.dram_tensor("xT", [D, BS], DT_QK, kind="ExternalInput")
    wqT_d = nc.dram_tensor("wqT", [D, ES], DT_QK, kind="ExternalInput")
    wkT_d = nc.dram_tensor("wkT", [D, ES], DT_QK, kind="ExternalInput")
    wvT_d = nc.dram_tensor("wvT", [D, ES], DT_V, kind="ExternalInput")
    bq_d = nc.dram_tensor("bq", [ES, 1], F32, kind="ExternalInput")
    bk_d = nc.dram_tensor("bk", [ES, 1], F32, kind="ExternalInput")
    bv_d = nc.dram_tensor("bv", [1, ES], F32, kind="ExternalInput")
    woT_d = nc.dram_tensor("woT", [ES, D], DT_OUT, kind="ExternalInput")
    y_d = nc.dram_tensor("y", [BS, D], F32, kind="ExternalOutput")
    if DEBUG:
        qT_dbg = nc.dram_tensor("qT_dbg", [P, BS], DT_QK, kind="ExternalOutput")
        kT_dbg = nc.dram_tensor("kT_dbg", [P, BS], DT_QK, kind="ExternalOutput")
        vA_dbg = nc.dram_tensor("vA_dbg", [P, N_CH * (HD + 1)], DT_ATT, kind="ExternalOutput")
        oT_dbg = nc.dram_tensor("oT_dbg", [P, BS], DT_OUT, kind="ExternalOutput")
        oraw_dbg = nc.dram_tensor("oraw_dbg", [HD + 1, 16 * NF], F32, kind="ExternalOutput")
        rcp_dbg = nc.dram_tensor("rcp_dbg", [1, 16 * NF], F32, kind="ExternalOutput")

    with tile.TileContext(nc) as tc:
        with tc.tile_pool(name="const", bufs=1) as cpool, \
             tc.tile_pool(name="xt", bufs=3) as xt_pool, \
             tc.tile_pool(name="qkv", bufs=1) as qkv_pool, \
             tc.tile_pool(name="pt", bufs=10) as pt_pool, \
             tc.tile_pool(name="ysb", bufs=4) as y_pool, \
             tc.tile_pool(name="ps", bufs=1, space="PSUM") as ps:

            # ---- constants / weights ------------------------------------
            wq_sb = cpool.tile([P, N_DC, ES], DT_QK)
            wk_sb = cpool.tile([P, N_DC, ES], DT_QK)
            wv_sb = cpool.tile([P, N_DC, ES], DT_V)
            nc.sync.dma_start(wq_sb[:], wqT_d.ap().rearrange("(a p) e -> p a e", p=P))
            nc.sync.dma_start(wk_sb[:], wkT_d.ap().rearrange("(a p) e -> p a e", p=P))
            nc.sync.dma_start(wv_sb[:], wvT_d.ap().rearrange("(a p) e -> p a e", p=P))
            wo_sb = cpool.tile([ES, D], DT_OUT)
            nc.sync.dma_start(wo_sb[:], woT_d[:])
            bq_sb = cpool.tile([ES, 1], F32)
            bk_sb = cpool.tile([ES, 1], F32)
            bv_row = cpool.tile([1, ES], F32)
            nc.sync.dma_start(bq_sb[:], bq_d[:])
            nc.sync.dma_start(bk_sb[:], bk_d[:])
            nc.sync.dma_start(bv_row[:], bv_d[:])
            ones_row = cpool.tile([1, ES], F32)
            nc.vector.memset(ones_row[:], 1.0)
            # ones at partition 64 for the recip-broadcast matmul (operands of
            # that matmul live on partition 64 = the rowsum row)
            ones_p64 = cpool.tile([HD + 1, HD], F32)
            nc.vector.memset(ones_p64[HD : HD + 1, :], 1.0)

            # bv broadcast to all 128 partitions via rank-1 matmul
            bv_bc_ps = ps.tile([P, ES], F32, tag="misc", bufs=2)
            nc.tensor.matmul(bv_bc_ps[:], ones_row[:], bv_row[:], start=True, stop=True)
            bv_bc = cpool.tile([P, ES], F32)
            nc.vector.tensor_copy(bv_bc[:], bv_bc_ps[:])

            # ---- persistent activations ---------------------------------
            qT_sb = qkv_pool.tile([P, BS], DT_QK)     # [feat 128, s 4096]
            kT_sb = qkv_pool.tile([P, BS], DT_QK)
            vA_sb = qkv_pool.tile([P, N_CH, HD + 1], DT_ATT)  # head A V + ones col
            vB_sb = qkv_pool.tile([P, N_CH, HD + 1], DT_ATT)
            oT_sb = qkv_pool.tile([P, BS], DT_OUT)    # normalized attn out, [feat, s]
            nc.vector.memset(vA_sb[:, :, HD : HD + 1], 1.0)
            nc.vector.memset(vB_sb[:, :, HD : HD + 1], 1.0)

            xT_r = xT_d.ap().rearrange("(a p) s -> p a s", p=P)

            # ---- emission helpers ---------------------------------------
            strips = {}

            def emit_strip_dma(sb):
                strip = xt_pool.tile([P, N_DC, NF], DT_QK, tag="strip", name=f"strip{sb}")
                nc.sync.dma_start(strip[:], xT_r[:, :, sb * NF : (sb + 1) * NF])
                strips[sb] = strip

            def emit_qk_piece(sb, which):
                s0 = sb * NF
                strip = strips[sb]
                w_sb, bias, dst = ((wq_sb, bq_sb, qT_sb) if which == "q"
                                   else (wk_sb, bk_sb, kT_sb))
                p_ps = ps.tile([P, NF], F32, tag="misc", bufs=2, name=f"{which}{sb}_ps")
                for j in range(N_DC):
                    nc.tensor.matmul(p_ps[:], w_sb[:, j], strip[:, j],
                                     start=(j == 0), stop=(j == N_DC - 1))
                nc.vector.tensor_scalar_add(dst[:, s0 : s0 + NF], p_ps[:], bias[:])

            def emit_v_piece(sb, ss):
                strip = strips[sb]
                ch = sb * (NF // P) + ss
                v_ps = ps.tile([P, ES], F32, tag="misc", bufs=2, name=f"v{ch}_ps")
                for j in range(N_DC):
                    nc.tensor.matmul(v_ps[:], strip[:, j, ss * P : (ss + 1) * P],
                                     wv_sb[:, j],
                                     start=(j == 0), stop=(j == N_DC - 1))
                nc.vector.tensor_add(vA_sb[:, ch, 0:HD], v_ps[:, 0:HD], bv_bc[:, 0:HD])
                nc.vector.tensor_add(vB_sb[:, ch, 0:HD], v_ps[:, HD:ES], bv_bc[:, HD:ES])

            def emit_sblock(sb):
                emit_strip_dma(sb)
                emit_qk_piece(sb, "q")
                emit_qk_piece(sb, "k")
                for ss in range(NF // P):
                    emit_v_piece(sb, ss)

            inv_sqrt_hd = 1.0 / float(np.sqrt(HD))
            y_queue = []

            def emit_recip_chain(oA_raw, oB_raw, q0, didx0):
                # 1/rowsum as exp(-ln(rowsum)) on ACT (2 ULP; the DVE
                # reciprocal takes 3.3us/call and stalls the PE), then ONE
                # partition-broadcast for both heads on the idle GPSIMD
                lg2 = pt_pool.tile([1, 2, NF], F32, tag="lg", bufs=4)
                rcp2 = pt_pool.tile([1, 2, NF], F32, tag="rcp", bufs=4)
                for hidx, o_raw in enumerate((oA_raw, oB_raw)):
                    nc.scalar.activation(lg2[:, hidx], o_raw[HD : HD + 1, :],
                                         mybir.ActivationFunctionType.Ln)
                    nc.scalar.activation(rcp2[:, hidx], lg2[:, hidx],
                                         mybir.ActivationFunctionType.Exp,
                                         scale=-1.0)
                    if DEBUG:
                        dsl = slice((didx0 + hidx) * NF, (didx0 + hidx + 1) * NF)
                        nc.sync.dma_start(oraw_dbg[:, dsl], o_raw[:])
                        nc.sync.dma_start(rcp_dbg[:, dsl], rcp2[:, hidx])
                bc2 = pt_pool.tile([HD, 2, NF], F32, tag="bc", bufs=3)
                nc.gpsimd.partition_broadcast(bc2[:], rcp2[:])
                return (oA_raw, oB_raw, bc2, q0)

            def emit_apply(oA_raw, oB_raw, bc2, q0):
                for hidx, (o_raw, part) in enumerate(((oA_raw, 0), (oB_raw, HD))):
                    nc.vector.tensor_mul(
                        oT_sb[part : part + HD, q0 : q0 + NF],
                        o_raw[0:HD, :], bc2[:, hidx])
                for ss in range(NF // P):
                    for ec in range(D // NF):
                        y_queue.append((q0 + ss * P, ec))

            def emit_yproj(s0, ec):
                y_ps = ps.tile([P, NF], F32, tag="misc", bufs=2)
                nc.tensor.matmul(y_ps[:], oT_sb[:, s0 : s0 + P],
                                 wo_sb[:, ec * NF : (ec + 1) * NF],
                                 start=True, stop=True)
                y_sb = y_pool.tile([P, NF], F32, tag="y")
                nc.vector.tensor_copy(y_sb[:], y_ps[:])
                nc.sync.dma_start(y_d[s0 : s0 + P, ec * NF : (ec + 1) * NF], y_sb[:])

            # ---- projections for batch 0 (k/v first; q trails as filler) -
            for sb in range(N_SB // 2):
                emit_strip_dma(sb)
                emit_qk_piece(sb, "k")
                for ss in range(NF // P):
                    emit_v_piece(sb, ss)
                if sb == 0:
                    emit_qk_piece(0, "q")

            # filler work queues: remaining q pieces + batch-1 projections
            # drip-feed into batch-0 attention; deferred output projections
            # drip into batch-1. q_sb{i} must complete before (b0, qc=i).
            a_queue = [("q", 1), ("q", 2), ("q", 3)]
            for sb in range(N_SB // 2, N_SB):
                a_queue.append(("dma", sb))
                a_queue.append(("q", sb))
                a_queue.append(("k", sb))
                for ss in range(NF // P):
                    a_queue.append(("v", sb, ss))

            pending = None

            def emit_a_piece():
                piece = a_queue.pop(0)
                if piece[0] == "dma":
                    emit_strip_dma(piece[1])
                    if a_queue:
                        emit_a_piece()  # dma is async; also emit a compute piece
                elif piece[0] in ("q", "k"):
                    emit_qk_piece(piece[1], piece[0])
                else:
                    emit_v_piece(piece[1], piece[2])

            # ---- attention: one continuous software pipeline -------------
            # Global stream over 64 ST pair-slots (8 per (b,qc) iteration);
            # PV consumption lags ST/exp by one pair and crosses iteration
            # boundaries, so the PE pipeline never drains mid-kernel.
            n_iters = B * N_QC
            total_pairs = n_iters * N_KP
            o_tiles = {}
            ptq = {}
            pending = None
            norm_state = None

            for g in range(total_pairs + 1):
                if g < total_pairs:
                    it = g // N_KP
                    kp = g % N_KP
                    b, qc = it // N_QC, it % N_QC
                    if kp == 0 and b == 1 and qc == 0:
                        while a_queue:
                            emit_a_piece()
                    q0 = b * S + qc * NF
                    st2A = ps.tile([P, 2, NF], F32, tag="st2", bufs=2)
                    st2B = ps.tile([P, 2, NF], F32, tag="st2", bufs=2)
                    for half in range(2):
                        k0 = b * S + (kp * 2 + half) * P
                        nc.tensor.matmul(st2A[:, half], kT_sb[0:HD, k0 : k0 + P],
                                         qT_sb[0:HD, q0 : q0 + NF],
                                         start=True, stop=True)
                        nc.tensor.matmul(st2B[:, half], kT_sb[HD:P, k0 : k0 + P],
                                         qT_sb[HD:P, q0 : q0 + NF],
                                         start=True, stop=True)
                    pt2A = pt_pool.tile([P, 2, NF], DT_ATT, tag="pt", bufs=10)
                    pt2B = pt_pool.tile([P, 2, NF], DT_ATT, tag="pt", bufs=10)
                    nc.scalar.activation(pt2A[:], st2A[:],
                                         mybir.ActivationFunctionType.Exp,
                                         scale=inv_sqrt_hd)
                    nc.scalar.activation(pt2B[:], st2B[:],
                                         mybir.ActivationFunctionType.Exp,
                                         scale=inv_sqrt_hd)
                    ptq[g] = (pt2A, pt2B)

                    # fillers ride the ST side of the stream
                    if b == 0:
                        if a_queue:
                            emit_a_piece()
                    else:
                        for _ in range(2):
                            if y_queue:
                                emit_yproj(*y_queue.pop(0))
                    if kp == 1 and pending is not None:
                        norm_state = emit_recip_chain(*pending)
                        pending = None
                    if kp == 4 and norm_state is not None:
                        emit_apply(*norm_state)
                        norm_state = None

                if g >= 1:
                    pg = g - 1
                    it = pg // N_KP
                    kp = pg % N_KP
                    b, qc = it // N_QC, it % N_QC
                    q0 = b * S + qc * NF
                    if kp == 0:
                        oA_new = ps.tile([HD + 1, NF], F32, tag="o", bufs=2)
                        oB_new = ps.tile([HD + 1, NF], F32, tag="o", bufs=2)
                        o_tiles[it] = (oA_new, oB_new)
                    oA_ps, oB_ps = o_tiles[it]
                    pt2A, pt2B = ptq.pop(pg)
                    for half in range(2):
                        kb = kp * 2 + half
                        gkb = b * N_KB + kb
                        nc.tensor.matmul(oA_ps[:], vA_sb[:, gkb], pt2A[:, half],
                                         start=(kb == 0), stop=(kb == N_KB - 1))
                        nc.tensor.matmul(oB_ps[:], vB_sb[:, gkb], pt2B[:, half],
                                         start=(kb == 0), stop=(kb == N_KB - 1))
                    if kp == N_KP - 1:
                        # iteration finished: free the o banks, defer norm
                        oA_raw = pt_pool.tile([HD + 1, NF], F32, tag="oraw", bufs=4)
                        nc.vector.tensor_copy(oA_raw[:], oA_ps[:])
                        oB_raw = pt_pool.tile([HD + 1, NF], F32, tag="oraw", bufs=4)
                        nc.vector.tensor_copy(oB_raw[:], oB_ps[:])
                        del o_tiles[it]
                        pending = (oA_raw, oB_raw, q0, it * 2)

            emit_apply(*emit_recip_chain(*pending))
            for s0, ec in y_queue:
                emit_yproj(s0, ec)

            if DEBUG:
                nc.sync.dma_start(qT_dbg[:], qT_sb[:])
                nc.sync.dma_start(kT_dbg[:], kT_sb[:])
                nc.sync.dma_start(vA_dbg[:], vA_sb.rearrange("p a e -> p (a e)"))
                nc.sync.dma_start(oT_dbg[:], oT_sb[:])

    nc.compile()
    return nc


def kernel(x, Wq, bq, Wk, bk, Wv, bv, Wo, bo, _trace=False):
    global last_results
    x = np.asarray(x, dtype=np.float32)
    Wq, bq = np.asarray(Wq, np.float32), np.asarray(bq, np.float32)
    Wk, bk = np.asarray(Wk, np.float32), np.asarray(bk, np.float32)
    Wv, bv = np.asarray(Wv, np.float32), np.asarray(bv, np.float32)
    Wo, bo = np.asarray(Wo, np.float32), np.asarray(bo, np.float32)

    if "nc" not in _cache:
        _cache["nc"] = _build()
    nc = _cache["nc"]

    dt_qk, dt_v, dt_out = _np_dt(DT_QK), _np_dt(DT_V), _np_dt(DT_OUT)
    xT = np.ascontiguousarray(x.reshape(BS, D).T)
    xT_qk = xT.astype(dt_qk, copy=False)
    in_maps = []
    for c in range(NCORES):
        sl = slice(c * ES, (c + 1) * ES)
        in_maps.append({
            "xT": xT_qk,
            "wqT": np.ascontiguousarray(Wq[sl].T).astype(dt_qk, copy=False),
            "wkT": np.ascontiguousarray(Wk[sl].T).astype(dt_qk, copy=False),
            "wvT": np.ascontiguousarray(Wv[sl].T).astype(dt_v, copy=False),
            "bq": np.ascontiguousarray(bq[sl, None]),
            "bk": np.ascontiguousarray(bk[sl, None]),
            "bv": np.ascontiguousarray(bv[None, sl]),
            "woT": np.ascontiguousarray(Wo[:, sl].T).astype(dt_out, copy=False),
        })

    res = bass_utils.run_bass_kernel_spmd(
        nc, in_maps, core_ids=list(range(NCORES)), trace=_trace)
    last_results = res

    y = res.results[0]["y"].astype(np.float64)
    for c in range(1, NCORES):
        y += res.results[c]["y"]
    y = (y + bo).astype(np.float32)
    return y.reshape(B, S, D)


# revision 31
# speedup vs baseline: 1.0240x; 1.0240x over previous
"""Chunked (= full, non-causal) multi-head self-attention on 8 TRN2 NeuronCores.

Problem: B=2, S=2048, D=1024, H=16 heads (head_dim 64), torch-Linear-style
projections (y = x @ W.T + b), softmax attention, output projection.

Sharding: head-parallel. Core c owns heads {2c, 2c+1} = feature slice
[128c, 128c+128). Each core computes q/k/v for its slice from the full x
(replicated), runs attention for its 4 (batch, head) pairs, and produces a
partial output projection with its 128-row slice of Wo. Host sums the 8
partials and adds bo.

Layout: scores are computed transposed, ST[k, q] (keys on partitions), so the
softmax exp output PT feeds the P@V matmul directly (contraction over k on
partitions) with no on-chip transposes anywhere — x and the weights are
pre-transposed on the host. The two heads' K=64 score matmuls land on PE
row-groups 0-1/2-3. The softmax denominator rides as row 64 of the PV output
via a ones-column appended to V (M=65); normalization is a reciprocal +
rank-1 broadcast matmul + one DVE multiply on the small [64, S] output,
deferred one iteration so it never stalls the PE.

Scheduling: the kernel is emitted as one software-pipelined stream designed to
keep the PE densely busy (TRN2's HAM clock-gate halves the PE clock if it ever
looks idle): projections for batch 0 first, then batch-0 attention interleaved
with batch-1 projections, then batch-1 attention interleaved with the deferred
output projections. Score tiles are written in 2-bank PSUM pairs so each exp
ACTIVATE covers 1024 columns, amortizing ACT's fixed PSUM-access latency.

Precision: everything bf16 in, fp32 accumulate/out (~5e-3 rel err).
"""

import sys

if "/opt/trn_rl_repo" not in sys.path:
    sys.path.insert(0, "/opt/trn_rl_repo")

import numpy as np

import concourse.bacc as bacc
import concourse.mybir as mybir
import concourse.tile as tile
from concourse import bass_utils

# Route Exp to the activation-table set that also holds Ln, so the softmax
# exps and the reciprocal-via-exp(-ln(x)) trick share one table (the default
# per-function choice would ping-pong table loads at ~2.7us each).
_orig_get_activation_tables = bacc.get_activation_tables


def _patched_get_activation_tables(arch):
    out = {}
    for name, funcs in dict(_orig_get_activation_tables(arch)).items():
        if name != "natural_log_exp_and_others":
            funcs = {f for f in funcs if f != mybir.ActivationFunctionType.Exp}
        out[name] = funcs
    return out


bacc.get_activation_tables = _patched_get_activation_tables

B, S, D, H = 2, 2048, 1024, 16
HD = D // H          # 64
NCORES = 8
ES = D // NCORES     # 128 features (= 2 heads) per core
BS = B * S           # 4096 rows total

P = 128              # partitions
NF = 512             # matmul free-dim tile
N_SB = BS // NF      # 8 s-blocks of 512
N_DC = D // P        # 8 contraction chunks of 128
N_KB = S // P        # 16 key blocks of 128 per batch
N_KP = N_KB // 2     # 8 key-block PAIRS per batch
N_QC = S // NF       # 4 query chunks of 512 per batch
N_CH = BS // P       # 32 global 128-row chunks

F32 = mybir.dt.float32
BF16 = mybir.dt.bfloat16

DT_QK = BF16         # x/Wq/Wk inputs for q,k projections + score matmuls
DT_V = BF16          # x/Wv inputs for v projection
DT_ATT = BF16        # attention weights (exp output) and V in the P@V matmul
DT_OUT = BF16        # output projection inputs (OT, Wo)

DEBUG = False

_cache = {}
last_results = None          # test.py reads exec_time_ns off this


def _np_dt(dt):
    import ml_dtypes

    return np.dtype(ml_dtypes.bfloat16) if dt == mybir.dt.bfloat16 else np.dtype(np.float32)


def _build():
    nc = bacc.Bacc("TRN2", target_bir_lowering=False, debug=False)

    xT_d = nc.dram_tensor("xT", [D, BS], DT_QK, kind="ExternalInput")
    wqT_d = nc.dram_tensor("wqT", [D, ES], DT_QK, kind="ExternalInput")
    wkT_d = nc.dram_tensor("wkT", [D, ES], DT_QK, kind="ExternalInput")
    wvT_d = nc.dram_tensor("wvT", [D, ES], DT_V, kind="ExternalInput")
    bq_d = nc.dram_tensor("bq", [ES, 1], F32, kind="ExternalInput")
    bk_d = nc.dram_tensor("bk", [ES, 1], F32, kind="ExternalInput")
    bv_d = nc.dram_tensor("bv", [1, ES], F32, kind="ExternalInput")
    woT_d = nc.dram_tensor("woT", [ES, D], DT_OUT, kind="ExternalInput")
    y_d = nc.dram_tensor("y", [BS, D], F32, kind="ExternalOutput")
    if DEBUG:
        qT_dbg = nc.dram_tensor("qT_dbg", [P, BS], DT_QK, kind="ExternalOutput")
        kT_dbg = nc.dram_tensor("kT_dbg", [P, BS], DT_QK, kind="ExternalOutput")
        vA_dbg = nc.dram_tensor("vA_dbg", [P, N_CH * (HD + 1)], DT_ATT, kind="ExternalOutput")
        oT_dbg = nc.dram_tensor("oT_dbg", [P, BS], DT_OUT, kind="ExternalOutput")
        oraw_dbg = nc.dram_tensor("oraw_dbg", [HD + 1, 16 * NF], F32, kind="ExternalOutput")
        rcp_dbg = nc.dram_tensor("rcp_dbg", [1, 16 * NF], F32, kind="ExternalOutput")

    with tile.TileContext(nc) as tc:
        with tc.tile_pool(name="const", bufs=1) as cpool, \
             tc.tile_pool(name="xt", bufs=3) as xt_pool, \
             tc.tile_pool(name="qkv", bufs=1) as qkv_pool, \
             tc.tile_pool(name="pt", bufs=14) as pt_pool, \
             tc.tile_pool(name="ysb", bufs=6) as y_pool, \
             tc.tile_pool(name="ps", bufs=1, space="PSUM") as ps:

            # ---- constants / weights ------------------------------------
            wq_sb = cpool.tile([P, N_DC, ES], DT_QK)
            wk_sb = cpool.tile([P, N_DC, ES], DT_QK)
            wv_sb = cpool.tile([P, N_DC, ES], DT_V)
            nc.sync.dma_start(wq_sb[:], wqT_d.ap().rearrange("(a p) e -> p a e", p=P))
            nc.sync.dma_start(wk_sb[:], wkT_d.ap().rearrange("(a p) e -> p a e", p=P))
            nc.sync.dma_start(wv_sb[:], wvT_d.ap().rearrange("(a p) e -> p a e", p=P))
            wo_sb = cpool.tile([ES, D], DT_OUT)
            nc.sync.dma_start(wo_sb[:], woT_d[:])
            bq_sb = cpool.tile([ES, 1], F32)
            bk_sb = cpool.tile([ES, 1], F32)
            bv_row = cpool.tile([1, ES], F32)
            nc.sync.dma_start(bq_sb[:], bq_d[:])
            nc.sync.dma_start(bk_sb[:], bk_d[:])
            nc.sync.dma_start(bv_row[:], bv_d[:])
            ones_row = cpool.tile([1, ES], F32)
            nc.vector.memset(ones_row[:], 1.0)
            # ones at partition 64 for the recip-broadcast matmul (operands of
            # that matmul live on partition 64 = the rowsum row)
            ones_p64 = cpool.tile([HD + 1, HD], F32)
            nc.vector.memset(ones_p64[HD : HD + 1, :], 1.0)

            # bv broadcast to all 128 partitions via rank-1 matmul
            bv_bc_ps = ps.tile([P, ES], F32, tag="misc", bufs=2)
            nc.tensor.matmul(bv_bc_ps[:], ones_row[:], bv_row[:], start=True, stop=True)
            bv_bc = cpool.tile([P, ES], F32)
            nc.vector.tensor_copy(bv_bc[:], bv_bc_ps[:])

            # ---- persistent activations ---------------------------------
            qT_sb = qkv_pool.tile([P, BS], DT_QK)     # [feat 128, s 4096]
            kT_sb = qkv_pool.tile([P, BS], DT_QK)
            vA_sb = qkv_pool.tile([P, N_CH, HD + 1], DT_ATT)  # head A V + ones col
            vB_sb = qkv_pool.tile([P, N_CH, HD + 1], DT_ATT)
            oT_sb = qkv_pool.tile([P, BS], DT_OUT)    # normalized attn out, [feat, s]
            nc.vector.memset(vA_sb[:, :, HD : HD + 1], 1.0)
            nc.vector.memset(vB_sb[:, :, HD : HD + 1], 1.0)

            xT_r = xT_d.ap().rearrange("(a p) s -> p a s", p=P)

            # ---- emission helpers ---------------------------------------
            strips = {}

            def emit_strip_dma(sb):
                strip = xt_pool.tile([P, N_DC, NF], DT_QK, tag="strip", name=f"strip{sb}")
                nc.sync.dma_start(strip[:], xT_r[:, :, sb * NF : (sb + 1) * NF])
                strips[sb] = strip

            def emit_qk_piece(sb, which):
                s0 = sb * NF
                strip = strips[sb]
                w_sb, bias, dst = ((wq_sb, bq_sb, qT_sb) if which == "q"
                                   else (wk_sb, bk_sb, kT_sb))
                p_ps = ps.tile([P, NF], F32, tag="misc", bufs=2, name=f"{which}{sb}_ps")
                for j in range(N_DC):
                    nc.tensor.matmul(p_ps[:], w_sb[:, j], strip[:, j],
                                     start=(j == 0), stop=(j == N_DC - 1))
                nc.vector.tensor_scalar_add(dst[:, s0 : s0 + NF], p_ps[:], bias[:])

            def emit_v_piece(sb, ss):
                strip = strips[sb]
                ch = sb * (NF // P) + ss
                v_ps = ps.tile([P, ES], F32, tag="misc", bufs=2, name=f"v{ch}_ps")
                for j in range(N_DC):
                    nc.tensor.matmul(v_ps[:], strip[:, j, ss * P : (ss + 1) * P],
                                     wv_sb[:, j],
                                     start=(j == 0), stop=(j == N_DC - 1))
                nc.vector.tensor_add(vA_sb[:, ch, 0:HD], v_ps[:, 0:HD], bv_bc[:, 0:HD])
                nc.vector.tensor_add(vB_sb[:, ch, 0:HD], v_ps[:, HD:ES], bv_bc[:, HD:ES])

            def emit_sblock(sb):
                emit_strip_dma(sb)
                emit_qk_piece(sb, "q")
                emit_qk_piece(sb, "k")
                for ss in range(NF // P):
                    emit_v_piece(sb, ss)

            inv_sqrt_hd = 1.0 / float(np.sqrt(HD))
            y_queue = []

            def emit_recip_chain(oA_raw, oB_raw, q0, didx0):
                # 1/rowsum as exp(-ln(rowsum)) on ACT (2 ULP; the DVE
                # reciprocal takes 3.3us/call and stalls the PE), then ONE
                # partition-broadcast for both heads on the idle GPSIMD
                lg2 = pt_pool.tile([1, 2, NF], F32, tag="lg", bufs=4)
                rcp2 = pt_pool.tile([1, 2, NF], F32, tag="rcp", bufs=4)
                for hidx, o_raw in enumerate((oA_raw, oB_raw)):
                    nc.scalar.activation(lg2[:, hidx], o_raw[HD : HD + 1, :],
                                         mybir.ActivationFunctionType.Ln)
                    nc.scalar.activation(rcp2[:, hidx], lg2[:, hidx],
                                         mybir.ActivationFunctionType.Exp,
                                         scale=-1.0)
                    if DEBUG:
                        dsl = slice((didx0 + hidx) * NF, (didx0 + hidx + 1) * NF)
                        nc.sync.dma_start(oraw_dbg[:, dsl], o_raw[:])
                        nc.sync.dma_start(rcp_dbg[:, dsl], rcp2[:, hidx])
                bc2 = pt_pool.tile([HD, 2, NF], F32, tag="bc", bufs=3)
                nc.gpsimd.partition_broadcast(bc2[:], rcp2[:])
                return (oA_raw, oB_raw, bc2, q0)

            def emit_apply(oA_raw, oB_raw, bc2, q0):
                for hidx, (o_raw, part) in enumerate(((oA_raw, 0), (oB_raw, HD))):
                    nc.vector.tensor_mul(
                        oT_sb[part : part + HD, q0 : q0 + NF],
                        o_raw[0:HD, :], bc2[:, hidx])
                for ss in range(NF // P):
                    for ec in range(D // NF):
                        y_queue.append((q0 + ss * P, ec))

            def emit_yproj(s0, ec):
                y_ps = ps.tile([P, NF], F32, tag="misc", bufs=2)
                nc.tensor.matmul(y_ps[:], oT_sb[:, s0 : s0 + P],
                                 wo_sb[:, ec * NF : (ec + 1) * NF],
                                 start=True, stop=True)
                y_sb = y_pool.tile([P, NF], F32, tag="y")
                nc.vector.tensor_copy(y_sb[:], y_ps[:])
                nc.sync.dma_start(y_d[s0 : s0 + P, ec * NF : (ec + 1) * NF], y_sb[:])

            # ---- projections for batch 0 (k/v first; q trails as filler) -
            for sb in range(N_SB // 2):
                emit_strip_dma(sb)
                emit_qk_piece(sb, "k")
                for ss in range(NF // P):
                    emit_v_piece(sb, ss)
                if sb == 0:
                    emit_qk_piece(0, "q")

            # filler work queues: remaining q pieces + batch-1 projections
            # drip-feed into batch-0 attention; deferred output projections
            # drip into batch-1. q_sb{i} must complete before (b0, qc=i).
            a_queue = [("q", 1), ("q", 2), ("q", 3)]
            for sb in range(N_SB // 2, N_SB):
                a_queue.append(("dma", sb))
                a_queue.append(("q", sb))
                a_queue.append(("k", sb))
                for ss in range(NF // P):
                    a_queue.append(("v", sb, ss))

            pending = None

            def emit_a_piece():
                piece = a_queue.pop(0)
                if piece[0] == "dma":
                    emit_strip_dma(piece[1])
                    if a_queue:
                        emit_a_piece()  # dma is async; also emit a compute piece
                elif piece[0] in ("q", "k"):
                    emit_qk_piece(piece[1], piece[0])
                else:
                    emit_v_piece(piece[1], piece[2])

            # ---- attention: one continuous software pipeline -------------
            # Global stream over 64 ST pair-slots (8 per (b,qc) iteration);
            # PV consumption lags ST/exp by one pair and crosses iteration
            # boundaries, so the PE pipeline never drains mid-kernel.
            n_iters = B * N_QC
            total_pairs = n_iters * N_KP
            o_tiles = {}
            ptq = {}
            pending = None
            norm_state = None

            for g in range(total_pairs + 1):
                if g < total_pairs:
                    it = g // N_KP
                    kp = g % N_KP
                    b, qc = it // N_QC, it % N_QC
                    if kp == 0 and b == 1 and qc == 0:
                        while a_queue:
                            emit_a_piece()
                    q0 = b * S + qc * NF
                    st2A = ps.tile([P, 2, NF], F32, tag="st2", bufs=2)
                    st2B = ps.tile([P, 2, NF], F32, tag="st2", bufs=2)
                    for half in range(2):
                        k0 = b * S + (kp * 2 + half) * P
                        nc.tensor.matmul(st2A[:, half], kT_sb[0:HD, k0 : k0 + P],
                                         qT_sb[0:HD, q0 : q0 + NF],
                                         start=True, stop=True)
                        nc.tensor.matmul(st2B[:, half], kT_sb[HD:P, k0 : k0 + P],
                                         qT_sb[HD:P, q0 : q0 + NF],
                                         start=True, stop=True)
                    pt2A = pt_pool.tile([P, 2, NF], DT_ATT, tag="pt", bufs=14)
                    pt2B = pt_pool.tile([P, 2, NF], DT_ATT, tag="pt", bufs=14)
                    nc.scalar.activation(pt2A[:], st2A[:],
                                         mybir.ActivationFunctionType.Exp,
                                         scale=inv_sqrt_hd)
                    nc.scalar.activation(pt2B[:], st2B[:],
                                         mybir.ActivationFunctionType.Exp,
                                         scale=inv_sqrt_hd)
                    ptq[g] = (pt2A, pt2B)

                    # fillers ride the ST side of the stream
                    if b == 0:
                        if a_queue:
                            emit_a_piece()
                    else:
                        npop = 3 if it == n_iters - 1 else 2
                        for _ in range(npop):
                            if y_queue:
                                emit_yproj(*y_queue.pop(0))
                    if kp == 1 and pending is not None:
                        norm_state = emit_recip_chain(*pending)
                        pending = None
                    if kp == 4 and norm_state is not None:
                        emit_apply(*norm_state)
                        norm_state = None

                if g >= 1:
                    pg = g - 1
                    it = pg // N_KP
                    kp = pg % N_KP
                    b, qc = it // N_QC, it % N_QC
                    q0 = b * S + qc * NF
                    if kp == 0:
                        oA_new = ps.tile([HD + 1, NF], F32, tag="o", bufs=2)
                        oB_new = ps.tile([HD + 1, NF], F32, tag="o", bufs=2)
                        o_tiles[it] = (oA_new, oB_new)
                    oA_ps, oB_ps = o_tiles[it]
                    pt2A, pt2B = ptq.pop(pg)
                    for half in range(2):
                        kb = kp * 2 + half
                        gkb = b * N_KB + kb
                        nc.tensor.matmul(oA_ps[:], vA_sb[:, gkb], pt2A[:, half],
                                         start=(kb == 0), stop=(kb == N_KB - 1))
                        nc.tensor.matmul(oB_ps[:], vB_sb[:, gkb], pt2B[:, half],
                                         start=(kb == 0), stop=(kb == N_KB - 1))
                    if kp == N_KP - 1:
                        # iteration finished: free the o banks, defer norm
                        oA_raw = pt_pool.tile([HD + 1, NF], F32, tag="oraw", bufs=6)
                        nc.vector.tensor_copy(oA_raw[:], oA_ps[:])
                        oB_raw = pt_pool.tile([HD + 1, NF], F32, tag="oraw", bufs=6)
                        nc.vector.tensor_copy(oB_raw[:], oB_ps[:])
                        del o_tiles[it]
                        pending = (oA_raw, oB_raw, q0, it * 2)

            emit_apply(*emit_recip_chain(*pending))
            for s0, ec in y_queue:
                emit_yproj(s0, ec)

            if DEBUG:
                nc.sync.dma_start(qT_dbg[:], qT_sb[:])
                nc.sync.dma_start(kT_dbg[:], kT_sb[:])
                nc.sync.dma_start(vA_dbg[:], vA_sb.rearrange("p a e -> p (a e)"))
                nc.sync.dma_start(oT_dbg[:], oT_sb[:])

    nc.compile()
    return nc


def kernel(x, Wq, bq, Wk, bk, Wv, bv, Wo, bo, _trace=False):
    global last_results
    x = np.asarray(x, dtype=np.float32)
    Wq, bq = np.asarray(Wq, np.float32), np.asarray(bq, np.float32)
    Wk, bk = np.asarray(Wk, np.float32), np.asarray(bk, np.float32)
    Wv, bv = np.asarray(Wv, np.float32), np.asarray(bv, np.float32)
    Wo, bo = np.asarray(Wo, np.float32), np.asarray(bo, np.float32)

    if "nc" not in _cache:
        _cache["nc"] = _build()
    nc = _cache["nc"]

    dt_qk, dt_v, dt_out = _np_dt(DT_QK), _np_dt(DT_V), _np_dt(DT_OUT)
    xT = np.ascontiguousarray(x.reshape(BS, D).T)
    xT_qk = xT.astype(dt_qk, copy=False)
    in_maps = []
    for c in range(NCORES):
        sl = slice(c * ES, (c + 1) * ES)
        in_maps.append({
            "xT": xT_qk,
            "wqT": np.ascontiguousarray(Wq[sl].T).astype(dt_qk, copy=False),
            "wkT": np.ascontiguousarray(Wk[sl].T).astype(dt_qk, copy=False),
            "wvT": np.ascontiguousarray(Wv[sl].T).astype(dt_v, copy=False),
            "bq": np.ascontiguousarray(bq[sl, None]),
            "bk": np.ascontiguousarray(bk[sl, None]),
            "bv": np.ascontiguousarray(bv[None, sl]),
            "woT": np.ascontiguousarray(Wo[:, sl].T).astype(dt_out, copy=False),
        })

    res = bass_utils.run_bass_kernel_spmd(
        nc, in_maps, core_ids=list(range(NCORES)), trace=_trace)
    last_results = res

    y = res.results[0]["y"].astype(np.float64)
    for c in range(1, NCORES):
        y += res.results[c]["y"]
    y = (y + bo).astype(np.float32)
    return y.reshape(B, S, D)
